# revision 65
# baseline (speedup 1.0000x reference)
"""ArcMarginProduct distributed Trainium2 kernel (8 NeuronCores).

Strategy (classifier/tensor parallel along out_features, per sharding hint):
  - weight [100000, 512] is row-sharded across 8 cores: 12500 classes each,
    padded to 12544 = 98*128 rows (pad rows are 1.0, outputs discarded).
  - input [512, 512] and label [512] are replicated (label passed as
    precomputed per-core local index tensors).
  - Each core computes outT_i = (S * cos(norm(X), norm(W_i)))^T for its
    class shard (TRANSPOSED: classes on rows), plus the one-hot ArcFace
    margin for labels in its shard (gather W rows -> phi -> out2).
  - Host transposes + concatenates the 8 [12500, 512] blocks.

v21 pipeline per core (138us HW, vs 186us baseline):
  W arrives PRE-CAST to bf16 from the host (RNE, numerically identical
     to the f32->bf16 cast-DMA it replaces) -> halves the dominant HBM
     read (25.7 -> 12.8 MB/core).
  Ring discipline (the big lesson): ALL bulk DMA (X, W loads, out
     writes) goes on the ONE gpsimd SWDGE ring, in FIFO order
     X -> W singles -> W quads -> interleaved outs; the slow random-row
     fixup gathers are emitted only after every W load.  Anything on a
     second ring loses the SDMA engines' packet round-robin and starves
     the W stream; anything early in the FIFO that waits on far-future
     data head-of-line blocks the engines behind it.
  X: 8KB/partition contiguous descriptors (partition p holds batch rows
     4p..4p+3; the host unpermutes for free in its transpose-copy).
     Cast bf16 (DVE) + row sumsq (2 ACT squares from f32, 2 DVE stt
     from bf16) -> rsqrt chain -> scale by S/||x_n|| -> PE transpose.
     ACT warmup ops (DVE-memset source) preload the sqrt table at t~1us.
  W: 4 single-band groups then 4-band quads (16KB/partition descs);
     partition p holds consecutive weight rows.  Per-BAND stats
     (sumsq mostly DVE stt + recip DVE + sqrt ACT), emitted 2 bands
     ahead -- 1 band for a group's first band -- AFTER that band's
     transpose drains, so no FIFO entry ever waits on an unlanded quad.
     Band 0: per-chunk stats + W transposes pre-emitted ahead of the
     X chain so the first matmul is gated only by xt (~19us).
  MM: per chunk: 4 k-slice matmuls, lhsT = raw-W^T slice (stationary),
      rhs = scaled-X^T k-slice [128, 512] -> psumT [c=128, n=512].
      Drain = ACT scalar.mul by 1/||w_c|| (per-partition) -> bf16.
      PSUM: 5 matmul banks + 3 transpose banks.
  Fixup: indirect-gather W[label] rows (bf16), normalize, row-dot vs X,
      phi with threshold select, x30 -> out2; host scatters.
"""

import math
import sys
import types

import numpy as np

# ---------------- constants (must match reference.py) ----------------
S = 30.0
M = 0.5
COS_M = math.cos(M)
SIN_M = math.sin(M)
TH = math.cos(math.pi - M)
MM = math.sin(math.pi - M) * M

N = 512          # batch
D = 512          # feature dim
C = 100000       # classes
N_CORES = 8
C_PER = C // N_CORES          # 12500
P = 128

_cache = {}


def _ensure_ntff_hook():
    """Install the axon NTFF profiling hook plumbing if this image's antenv
    lacks it (lets run_bass_kernel_spmd(trace=True) return exec_time_ns)."""
    try:
        import antenv.axon_hooks  # noqa: F401
        return
    except ImportError:
        pass
    import antenv
    m = types.ModuleType("antenv.axon_hooks")
    _hook = [None]
    m.set_axon_ntff_profile_hook = lambda h: _hook.__setitem__(0, h)
    m.get_axon_ntff_profile_hook = lambda: _hook[0]
    sys.modules["antenv.axon_hooks"] = m
    antenv.axon_hooks = m
    try:
        from trn_agent_boot.trn_boot import _ntff_profile_via_ctypes
        m.set_axon_ntff_profile_hook(
            _ntff_profile_via_ctypes("/opt/axon/libaxon_pjrt.so"))
    except Exception:
        pass


def build_nc(n_chunks=98, fixup="full", out_bf16=True):
    """Build the per-core Bass graph. n_chunks*128 = padded shard width."""
    from contextlib import ExitStack

    import concourse.bass as bass
    import concourse.tile as tile
    from concourse import bacc, mybir
    from concourse.masks import make_identity

    f32 = mybir.dt.float32
    bf16 = mybir.dt.bfloat16
    i32 = mybir.dt.int32
    A = mybir.AluOpType
    AF = mybir.ActivationFunctionType

    c_pad = n_chunks * P
    n_bands = (n_chunks + 3) // 4        # bands of up to 4 chunks

    nc = bacc.Bacc("TRN2", target_bir_lowering=False, debug=False,
                   num_devices=N_CORES)

    x_d = nc.dram_tensor("x", [N, D], f32, kind="ExternalInput")
    # W arrives PRE-CAST to bf16 (host-side RNE cast, numerically
    # identical to the f32->bf16 cast-DMA it replaces): halves the
    # dominant HBM read stream (25.7 MB -> 12.8 MB per core)
    w_d = nc.dram_tensor("w", [c_pad, D], bf16, kind="ExternalInput")
    gidx_d = nc.dram_tensor("gidx", [P, 4], i32, kind="ExternalInput")
    out_dt = bf16 if out_bf16 else f32
    # transposed output: row = class (shard-local), col = batch slot j
    # (j = a*128 + p  <->  batch row n = 4p + a; host unpermutes)
    out_d = nc.dram_tensor("out", [c_pad * N], out_dt, kind="ExternalOutput")
    out2_d = nc.dram_tensor("out2", [P, 4], f32, kind="ExternalOutput")

    outT = out_d.ap().rearrange("(c n) -> c n", n=N)

    # W DMA loads cover a GROUP of bands at a time; within a group
    # partition p holds apg CONSECUTIVE weight rows -> contiguous DMA on
    # both sides, and the transposed-output DMA lands rows directly in
    # class order. First NHW groups are single-band so the pipeline
    # fills fast.
    GBANDS = 4
    NHW = 4                           # leading single-band groups
    group_bands = [[i] for i in range(NHW)]
    bb = NHW
    while bb < n_bands - 2:
        group_bands.append(list(range(bb, min(bb + GBANDS, n_bands - 2))))
        bb += GBANDS
    for bb in range(max(NHW, n_bands - 2), n_bands):
        group_bands.append([bb])     # small tail groups drain fast
    n_groups = len(group_bands)
    band_to_group = {}
    for gi, bl in enumerate(group_bands):
        for bj in bl:
            band_to_group[bj] = gi

    def group_rows(g):
        bl = group_bands[g]
        r0 = bl[0] * 512
        last = min((bl[-1] + 1) * 4, n_chunks) * P
        return r0, last - r0

    with tile.TileContext(nc) as tc:
        with ExitStack() as ctx:
            const_p = ctx.enter_context(tc.tile_pool(name="const", bufs=1))
            xp = ctx.enter_context(tc.tile_pool(name="xp", bufs=1))
            wl_p = ctx.enter_context(tc.tile_pool(name="wl", bufs=4))
            wls_p = ctx.enter_context(tc.tile_pool(name="wls", bufs=4))
            wsc_p = ctx.enter_context(tc.tile_pool(name="wsc", bufs=6))
            wst_p = ctx.enter_context(tc.tile_pool(name="wst", bufs=8))
            wtb_p = ctx.enter_context(tc.tile_pool(name="wtb", bufs=6))
            ob_p = ctx.enter_context(tc.tile_pool(name="ob", bufs=3))
            fix_p = ctx.enter_context(tc.tile_pool(name="fix", bufs=1))
            ptr_p = ctx.enter_context(
                tc.tile_pool(name="ptr", bufs=3, space="PSUM"))
            pmm_p = ctx.enter_context(
                tc.tile_pool(name="pmm", bufs=5, space="PSUM"))

            # ---------------- X load ----------------
            # FIRST op on the gpsimd SWDGE ring: strict FIFO puts X's
            # descriptors ahead of every W load -> lands ~10us.  (On the
            # sync ring it loses the engine round-robin to the W stream
            # and lands ~17us.)
            # partition p <- batch rows 4p..4p+3 (contiguous 8KB descs)
            xin = xp.tile([P, 4 * D], dtype=f32)    # row a at cols a*512
            with tc.high_priority():
                nc.gpsimd.dma_start(
                    out=xin[:],
                    in_=x_d.ap().rearrange("(p a) d -> p (a d)", p=P))

            # ACT warmup: pull the sqrt/square table-set loads off the
            # critical path.  Source tile comes from a DVE memset (NOT
            # ident -- that is built late on the gpsimd queue and the
            # scheduler would defer the warmup behind it).
            actw = const_p.tile([P, 1], dtype=f32)
            nc.vector.memset(actw[:], 1.0)
            nc.scalar.activation(out=actw[:], in_=actw[:], func=AF.Square)
            nc.scalar.activation(out=actw[:], in_=actw[:], func=AF.Sqrt)

            # cast (DVE) runs first; row-sumsq split: a=0,1 on DVE (stt
            # from the bf16 copy), a=2,3 on ACT (Square from f32)
            xsc = xp.tile([P, 4 * D], dtype=bf16)   # S/||x|| * X, bf16
            xss = xp.tile([P, 4], dtype=f32)
            xqa = xp.tile([P, D], dtype=bf16)       # ACT square scratch
            xqs = xp.tile([P, D], dtype=bf16)       # DVE square scratch
            # sumsq split 2 ACT (from f32, parallel with the casts) +
            # 2 DVE (bf16 stt after the casts): shortest serial chain
            for a in range(4):
                nc.vector.tensor_copy(
                    xsc[:, a * D:(a + 1) * D], xin[:, a * D:(a + 1) * D])
            for a in range(1, 4):
                nc.scalar.activation(
                    out=xqa[:], in_=xin[:, a * D:(a + 1) * D],
                    func=AF.Square, accum_out=xss[:, a:a + 1])
            for a in range(1):
                xs = xsc[:, a * D:(a + 1) * D]
                nc.vector.scalar_tensor_tensor(
                    out=xqs[:], in0=xs, scalar=1.0, in1=xs,
                    op0=A.mult, op1=A.mult, accum_out=xss[:, a:a + 1])
            xrs = xp.tile([P, 4], dtype=f32)      # 1/sumsq
            xrn = xp.tile([P, 4], dtype=f32)      # 1/||x||   (fixup)
            xrnS = xp.tile([P, 4], dtype=f32)     # S/||x||
            nc.vector.reciprocal(out=xrs[:], in_=xss[:])
            nc.scalar.sqrt(out=xrn[:], in_=xrs[:])
            nc.scalar.activation(out=xrnS[:], in_=xrs[:], func=AF.Sqrt,
                                 scale=S * S)
            for a in range(4):
                nc.vector.tensor_scalar_mul(
                    xsc[:, a * D:(a + 1) * D],
                    xsc[:, a * D:(a + 1) * D], xrnS[:, a:a + 1])

            # ---------------- W load (SWDGE DMA, cast f32 -> bf16) ------
            groups = {}     # g -> wl tile [P, GBANDS*4*D] bf16

            def emit_load_group(g):
                r0, rows = group_rows(g)
                apg = rows // P          # consecutive rows per partition
                if apg <= 4:
                    wl = wls_p.tile([P, 4 * D], dtype=bf16, tag="wls",
                                    name=f"wl{g}")
                else:
                    wl = wl_p.tile([P, GBANDS * 4 * D], dtype=bf16,
                                   tag="wl", name=f"wl{g}")
                nc.gpsimd.dma_start(
                    out=wl[:, :apg * D],
                    in_=w_d.ap()[r0:r0 + rows, :]
                        .rearrange("(p a) d -> p (a d)", p=P))
                groups[g] = wl

            # ---------------- per-BAND norm stats ----------------------
            # sumsq mostly on DVE (bf16 stt); a slice on ACT for
            # balance.  Emitted per band, 2 bands ahead of use, AFTER
            # that band's transpose drains: a stats op never sits in an
            # engine FIFO ahead of nearer-term work while waiting on a
            # far-future DMA (head-of-line blocking).
            bstats = {}      # b -> (wss, wrs, wrn) tiles [P, 4] f32

            def _bstat_tiles(b):
                if b not in bstats:
                    bstats[b] = (
                        wst_p.tile([P, 4], dtype=f32, tag="wss",
                                   name=f"wss{b}"),
                        wst_p.tile([P, 4], dtype=f32, tag="wrs",
                                   name=f"wrs{b}"),
                        wst_p.tile([P, 4], dtype=f32, tag="wrn",
                                   name=f"wrn{b}"))
                return bstats[b]

            def stage1_chunk(b, s):
                """Sumsq+rsqrt for one chunk (fine-grained: fill phase)."""
                g = band_to_group[b]
                wl = groups[g]
                sg = (b - group_bands[g][0]) * 4 + s
                wss, wrs, wrn = _bstat_tiles(b)
                wsl = wl[:, sg * D:(sg + 1) * D]
                wsq = wsc_p.tile([P, D], dtype=bf16, tag="wsq",
                                 name=f"wsq{b}_{s}")
                # scalar=xt[:,0:1] with op0=bypass: numerically inert,
                # but makes the stat depend on xt so the scheduler can
                # never sort it ahead of the X chain in the DVE FIFO
                nc.vector.scalar_tensor_tensor(
                    out=wsq[:], in0=wsl, scalar=xt[:, 0:1], in1=wsl,
                    op0=A.bypass, op1=A.mult, accum_out=wss[:, s:s + 1])
                nc.vector.reciprocal(out=wrs[:, s:s + 1],
                                     in_=wss[:, s:s + 1])
                nc.scalar.activation(out=wrn[:, s:s + 1],
                                     in_=wrs[:, s:s + 1], func=AF.Sqrt)

            def stage1_band(b):
                g = band_to_group[b]
                wl = groups[g]
                goff = (b - group_bands[g][0]) * 4
                nsub = min((b + 1) * 4, n_chunks) - b * 4
                wss, wrs, wrn = _bstat_tiles(b)
                for s in range(nsub):
                    sg = goff + s
                    wsl = wl[:, sg * D:(sg + 1) * D]
                    if (b * 4 + s) % 8 != 7:
                        wsq = wsc_p.tile([P, D], dtype=bf16, tag="wsq",
                                         name=f"wsq{b}_{s}")
                        nc.vector.scalar_tensor_tensor(
                            out=wsq[:], in0=wsl, scalar=1.0,
                            in1=wsl, op0=A.mult, op1=A.mult,
                            accum_out=wss[:, s:s + 1])
                    else:
                        wsqa = wsc_p.tile([P, D], dtype=bf16, tag="wsqa",
                                          name=f"wsqa{b}_{s}")
                        nc.scalar.activation(
                            out=wsqa[:], in_=wsl, func=AF.Square,
                            accum_out=wss[:, s:s + 1])
                nc.vector.reciprocal(out=wrs[:, :nsub], in_=wss[:, :nsub])
                nc.scalar.activation(out=wrn[:, :nsub], in_=wrs[:, :nsub],
                                     func=AF.Sqrt)

            def emit_band_transposes(b, gwl, goff, nsub):
                """PE-transpose one band's W chunks -> k-major wtb tile."""
                wtb = wtb_p.tile([P, 4 * 512], dtype=bf16, tag="wtb",
                                 name=f"wtb{b}")
                for s0 in range(0, nsub, 2):
                    wtp = ptr_p.tile([P, 8 * P], dtype=bf16, space="PSUM",
                                     tag="tp")
                    for ds in range(2):
                        si = goff + s0 + ds
                        for k in range(4):
                            nc.tensor.transpose(
                                out=wtp[:, k * 2 * P + ds * P:
                                        k * 2 * P + (ds + 1) * P],
                                in_=gwl[:, si * D + k * P:
                                        si * D + (k + 1) * P],
                                identity=ident[:])
                    # drain psum -> band tile (k-major layout)
                    nc.vector.tensor_copy(
                        out=wtb[:].rearrange("p (k c) -> p k c", k=4)
                            [:, :, s0 * P:(s0 + 2) * P],
                        in_=wtp[:].rearrange("p (k c) -> p k c", k=4))
                return wtb

            ident = const_p.tile([P, P], dtype=bf16)
            make_identity(nc, ident[:])

            # singles + first quad onto the SWDGE ring
            for _g in range(NHW + 1):
                emit_load_group(_g)

            # band 0's W transposes ahead of the X transposes in the PE
            # FIFO: they run while X-prep is still on DVE/ACT, so the
            # first matmul is gated only by xt
            wtb0 = emit_band_transposes(0, groups[0], 0, 4)

            # XT: [d(part), k-major: k*512 + j] bf16 (scaled), j=a*128+p
            xt = xp.tile([P, 4 * N], dtype=bf16)
            for k in range(4):
                pk = ptr_p.tile([P, 4 * P], dtype=bf16, space="PSUM", tag="tp")
                for a in range(4):
                    nc.tensor.transpose(
                        out=pk[:, a * P:(a + 1) * P],
                        in_=xsc[:, a * D + k * P: a * D + (k + 1) * P],
                        identity=ident[:])
                # drain on ACT (idle after the X squares): shortens the
                # DVE-serial head chain by ~2us
                nc.scalar.mul(out=xt[:, k * N:(k + 1) * N], in_=pk[:],
                              mul=1.0)

            # ---------------- sparse margin fixup (emitted mid-stream) ---
            fixst = {"vals": None}

            def emit_fixup_a():
                gidx = fix_p.tile([P, 4], dtype=i32)
                nc.sync.dma_start(out=gidx[:], in_=gidx_d.ap())

                wg = fix_p.tile([P, 4 * D], dtype=bf16)
                if fixup != "nogather":
                    for g in range(4):
                        nc.gpsimd.indirect_dma_start(
                            out=wg[:, g * D:(g + 1) * D], out_offset=None,
                            in_=w_d.ap(),
                            in_offset=bass.IndirectOffsetOnAxis(
                                ap=gidx[:, g:g + 1], axis=0))
                else:
                    nc.gpsimd.memset(wg[:], 1.0)
                fixst["wg"] = wg

            def emit_fixup_b(g):
                wg = fixst["wg"]
                if g == 0:
                    fixst["st"] = fix_p.tile([P, 16], dtype=f32,
                                             name="fixstat")
                st = fixst["st"]
                sumsq = st[:, 0:4]
                wgsq = fix_p.tile([P, D], dtype=f32, tag="wgsq",
                                  name=f"wgsq{g}")
                nc.scalar.activation(out=wgsq[:],
                                     in_=wg[:, g * D:(g + 1) * D],
                                     func=AF.Square,
                                     accum_out=sumsq[:, g:g + 1])
                dsc = fix_p.tile([P, D], dtype=f32, tag="wgsq",
                                 name=f"dsc{g}")
                nc.vector.tensor_tensor(
                    out=dsc[:], in0=xin[:, g * D:(g + 1) * D],
                    in1=wg[:, g * D:(g + 1) * D], op=A.mult)
                nc.vector.tensor_reduce(
                    out=st[:, 12 + g:13 + g], in_=dsc[:],
                    axis=mybir.AxisListType.X, op=A.add)

            def emit_fixup():
                st = fixst["st"]
                sumsq = st[:, 0:4]
                rs = st[:, 4:8]
                rn = st[:, 8:12]
                nc.vector.reciprocal(out=rs[:], in_=sumsq[:])
                nc.scalar.sqrt(out=rn[:], in_=rs[:])       # 1/||w||
                dots = st[:, 12:16]

                ft = fix_p.tile([P, 4 * 8], dtype=f32)
                cosv, cos2, sine, phi, alt, _unused, fvals, tmp = (
                    ft[:, i * 4:(i + 1) * 4] for i in range(8))
                mask_t = fix_p.tile([P, 4], dtype=mybir.dt.uint8)
                mask = mask_t[:]
                nc.vector.tensor_tensor(out=cosv, in0=dots[:], in1=rn[:],
                                        op=A.mult)
                nc.vector.tensor_tensor(out=cosv, in0=cosv, in1=xrn[:],
                                        op=A.mult)
                nc.vector.tensor_tensor(out=cos2, in0=cosv, in1=cosv,
                                        op=A.mult)
                nc.vector.tensor_scalar_min(cos2, cos2, 1.0)
                nc.scalar.activation(out=sine, in_=cos2, func=AF.Sqrt,
                                     scale=-1.0, bias=1.0)
                nc.vector.tensor_scalar_mul(phi, cosv, COS_M)
                nc.vector.tensor_scalar_mul(tmp, sine, SIN_M)
                nc.vector.tensor_tensor(out=phi, in0=phi, in1=tmp,
                                        op=A.subtract)
                nc.vector.tensor_scalar_add(alt, cosv, -MM)
                nc.vector.tensor_scalar(out=mask, in0=cosv, scalar1=TH,
                                        scalar2=None, op0=A.is_gt)
                nc.vector.select(out=fvals, mask=mask, on_true=phi,
                                 on_false=alt)
                nc.vector.tensor_scalar_mul(fvals, fvals, S)
                nc.sync.dma_start(out=out2_d.ap(), in_=fvals)
                fixst["vals"] = fvals


            # ---------------- main band loop ----------------
            ost = {}       # g -> staging tile [P, 8*N] bf16
            prog = {"load": NHW + 1}

            for b in range(n_bands):
                g = band_to_group[b]
                t = b - group_bands[g][0]
                chunks = range(b * 4, min((b + 1) * 4, n_chunks))
                nsub = len(chunks)

                # prefetch: W DMA ~12 bands ahead
                while (prog["load"] < n_groups
                       and group_bands[prog["load"]][0] <= b + 12):
                    emit_load_group(prog["load"])
                    prog["load"] += 1
                if t == 0:
                    ost[g] = ob_p.tile([P, GBANDS * 4 * N], dtype=out_dt,
                                       tag="ost", name=f"ost{g}")
                # fixup AFTER all W loads are emitted (b=12): the gather
                # descriptors are slow random-row reads; placed mid-ring
                # they block the quad stream for ~10us+
                if fixup != "none" and n_bands > 22:
                    if b == 12:
                        emit_fixup_a()
                    if 16 <= b <= 19:
                        emit_fixup_b(b - 16)
                    elif b == 21:
                        emit_fixup()

                gwl = groups[g]
                goff = (b - group_bands[g][0]) * 4

                # transpose RAW W chunks (pairs share one PSUM tile);
                # band 0's were pre-emitted ahead of the X-prep chain
                if b == 0:
                    wtb = wtb0
                else:
                    wtb = emit_band_transposes(b, gwl, goff, nsub)

                # stats lookahead 2 bands, EXCEPT the first band of each
                # group: lookahead 1 (prev group's last band), so the
                # ACT sqrt never queues ahead of drains while its
                # group's DMA is still in flight (head-of-line block).
                # bands 0/1 get per-chunk stats inline below.
                for m in range(b + 1, min(b + 3, n_bands)):
                    if m < 2:
                        continue
                    trig = max(m - 2, group_bands[band_to_group[m]][0] - 1)
                    if trig == b:
                        stage1_band(m)
                wrn = _bstat_tiles(b)[2]

                # matmuls: psumT[c, n] per chunk, accumulate over k
                for s in range(nsub):
                    sg = t * 4 + s
                    if b < 2:
                        stage1_chunk(b, s)
                    pm = pmm_p.tile([P, N], dtype=f32, space="PSUM")
                    for k in range(4):
                        nc.tensor.matmul(
                            out=pm[:],
                            lhsT=wtb[:, k * 512 + s * P: k * 512 + (s + 1) * P],
                            rhs=xt[:, k * N:(k + 1) * N],
                            start=(k == 0), stop=(k == 3))
                    # drain with per-partition 1/||w_c|| scale (all ACT;
                    # DVE owns sumsq + transpose drains).  Tail singles:
                    # alternate ACT/DVE so the last band drains ~2x
                    # faster (shorter kernel tail).
                    if b >= n_bands - 2 and s % 2 == 1:
                        nc.vector.tensor_scalar_mul(
                            ost[g][:, sg * N:(sg + 1) * N],
                            pm[:], wrn[:, s:s + 1])
                    else:
                        nc.scalar.mul(
                            out=ost[g][:, sg * N:(sg + 1) * N],
                            in_=pm[:], mul=wrn[:, s:s + 1])

                last_quad = (g == n_groups - 3)
                if last_quad:
                    # last quad: out-DMA per BAND (0.5 MB as soon as
                    # each band drains) to shorten the kernel tail.
                    # Band t's class rows are r0+p*apg+(4t..4t+nsub):
                    # slice the group-rearranged AP by column range.
                    r0, rows = group_rows(g)
                    nc.gpsimd.dma_start(
                        out=outT[r0:r0 + rows, :]
                            .rearrange("(p a) n -> p (a n)", p=P)
                            [:, t * 4 * N:(t * 4 + nsub) * N],
                        in_=ost[g][:, t * 4 * N:(t * 4 + nsub) * N])
                    if b == group_bands[g][-1]:
                        del ost[g]
                elif b == group_bands[g][-1]:
                    r0, rows = group_rows(g)
                    apg = rows // P
                    # out-DMA on the SAME SWDGE ring as the W loads:
                    # on the sync ring it steals the engine round-robin
                    # from the W stream (~270 GB/s effective vs ~390)
                    nc.gpsimd.dma_start(
                        out=outT[r0:r0 + rows, :]
                            .rearrange("(p a) n -> p (a n)", p=P),
                        in_=ost[g][:, :apg * N])
                    del ost[g]

            # margin values for tiny configs (normally emitted mid-stream)
            if fixup != "none" and fixst["vals"] is None:
                emit_fixup_a()
                for g in range(4):
                    emit_fixup_b(g)
                emit_fixup()

    nc.compile()
    return nc


def make_in_maps(input, label, weight, n_chunks=98, c_per=C_PER):
    """Shard the full inputs into per-core input maps."""
    from ml_dtypes import bfloat16

    c_pad = n_chunks * P
    x = np.ascontiguousarray(input, dtype=np.float32)
    lab = np.asarray(label).astype(np.int64)
    w = np.asarray(weight, dtype=np.float32)
    in_maps = []
    for i in range(N_CORES):
        c0 = i * c_per
        # pre-cast to bf16 on host (RNE, same numerics as the cast-DMA
        # it replaces); halves the device's W read bytes
        wi = np.empty((c_pad, D), dtype=bfloat16)
        wi[:c_per] = w[c0:c0 + c_per].astype(bfloat16)
        wi[c_per:] = 1.0
        loc = lab - c0
        valid = (loc >= 0) & (loc < c_per)
        g_rows = np.where(valid, loc, 0).astype(np.int32)
        in_maps.append({
            "x": x,
            "w": wi,
            # device row (p, a) = batch row 4p+a
            "gidx": np.ascontiguousarray(g_rows.reshape(P, 4)),
        })
    return in_maps


def kernel(input, label, weight):
    """Full inputs in, full output out. Runs SPMD on 8 NeuronCores."""
    _ensure_ntff_hook()
    from concourse.bass_utils import run_bass_kernel_spmd

    if "nc" not in _cache:
        _cache["nc"] = build_nc()
    nc = _cache["nc"]

    in_maps = make_in_maps(input, label, weight)
    res = run_bass_kernel_spmd(nc, in_maps, list(range(N_CORES)))
    _cache["last_result"] = res

    c_pad = 98 * P
    # device output is transposed: [c_pad, N] rows in class order; its
    # column j = a*128+p holds batch row n = 4p+a -> gather cols in
    # batch order while transposing (one fused fancy-index copy)
    big = np.concatenate(
        [res.results[i]["out"].reshape(c_pad, N)[:C_PER, :]
         for i in range(N_CORES)], axis=0)
    n_idx = np.arange(N)
    j_of_n = (n_idx % 4) * P + n_idx // 4
    out = big.T[j_of_n].astype(np.float32)
    out = np.ascontiguousarray(out)
    # place the device-computed margin values at the label positions
    lab = np.asarray(label).astype(np.int64)
    rows = np.arange(N)
    for i in range(N_CORES):
        vals = np.asarray(res.results[i]["out2"]).reshape(N)  # [p,a]->4p+a
        sel = (lab >= i * C_PER) & (lab < (i + 1) * C_PER)
        out[rows[sel], lab[sel]] = vals[sel]
    return out


# revision 66
# speedup vs baseline: 1.1742x; 1.1742x over previous
"""ArcMarginProduct distributed Trainium2 kernel (8 NeuronCores).

Strategy (classifier/tensor parallel along out_features, per sharding hint):
  - weight [100000, 512] is row-sharded across 8 cores: 12500 classes each,
    padded to 12544 = 98*128 rows (pad rows are 1.0, outputs discarded).
  - input [512, 512] and label [512] are replicated (label passed as
    precomputed per-core local index tensors).
  - Each core computes outT_i = (S * cos(norm(X), norm(W_i)))^T for its
    class shard (TRANSPOSED: classes on rows), plus the one-hot ArcFace
    margin for labels in its shard (gather W rows -> phi -> out2).
  - Host transposes + concatenates the 8 [12500, 512] blocks.

v21 pipeline per core (138us HW, vs 186us baseline):
  W arrives PRE-CAST to bf16 from the host (RNE, numerically identical
     to the f32->bf16 cast-DMA it replaces) -> halves the dominant HBM
     read (25.7 -> 12.8 MB/core).
  Ring discipline (the big lesson): ALL bulk DMA (X, W loads, out
     writes) goes on the ONE gpsimd SWDGE ring, in FIFO order
     X -> W singles -> W quads -> interleaved outs; the slow random-row
     fixup gathers are emitted only after every W load.  Anything on a
     second ring loses the SDMA engines' packet round-robin and starves
     the W stream; anything early in the FIFO that waits on far-future
     data head-of-line blocks the engines behind it.
  X: 8KB/partition contiguous descriptors (partition p holds batch rows
     4p..4p+3; the host unpermutes for free in its transpose-copy).
     Cast bf16 (DVE) + row sumsq (2 ACT squares from f32, 2 DVE stt
     from bf16) -> rsqrt chain -> scale by S/||x_n|| -> PE transpose.
     ACT warmup ops (DVE-memset source) preload the sqrt table at t~1us.
  W: 4 single-band groups then 4-band quads (16KB/partition descs);
     partition p holds consecutive weight rows.  Per-BAND stats
     (sumsq mostly DVE stt + recip DVE + sqrt ACT), emitted 2 bands
     ahead -- 1 band for a group's first band -- AFTER that band's
     transpose drains, so no FIFO entry ever waits on an unlanded quad.
     Band 0: per-chunk stats + W transposes pre-emitted ahead of the
     X chain so the first matmul is gated only by xt (~19us).
  MM: per chunk: 4 k-slice matmuls, lhsT = raw-W^T slice (stationary),
      rhs = scaled-X^T k-slice [128, 512] -> psumT [c=128, n=512].
      Drain = ACT scalar.mul by 1/||w_c|| (per-partition) -> bf16.
      PSUM: 5 matmul banks + 3 transpose banks.
  Fixup: indirect-gather W[label] rows (bf16), normalize, row-dot vs X,
      phi with threshold select, x30 -> out2; host scatters.
"""

import math
import sys
import types

import numpy as np

# ---------------- constants (must match reference.py) ----------------
S = 30.0
M = 0.5
COS_M = math.cos(M)
SIN_M = math.sin(M)
TH = math.cos(math.pi - M)
MM = math.sin(math.pi - M) * M

N = 512          # batch
D = 512          # feature dim
C = 100000       # classes
N_CORES = 8
C_PER = C // N_CORES          # 12500
P = 128

_cache = {}


def _ensure_ntff_hook():
    """Install the axon NTFF profiling hook plumbing if this image's antenv
    lacks it (lets run_bass_kernel_spmd(trace=True) return exec_time_ns)."""
    try:
        import antenv.axon_hooks  # noqa: F401
        return
    except ImportError:
        pass
    import antenv
    m = types.ModuleType("antenv.axon_hooks")
    _hook = [None]
    m.set_axon_ntff_profile_hook = lambda h: _hook.__setitem__(0, h)
    m.get_axon_ntff_profile_hook = lambda: _hook[0]
    sys.modules["antenv.axon_hooks"] = m
    antenv.axon_hooks = m
    try:
        from trn_agent_boot.trn_boot import _ntff_profile_via_ctypes
        m.set_axon_ntff_profile_hook(
            _ntff_profile_via_ctypes("/opt/axon/libaxon_pjrt.so"))
    except Exception:
        pass


def build_nc(n_chunks=98, fixup="full", out_bf16=True):
    """Build the per-core Bass graph. n_chunks*128 = padded shard width."""
    from contextlib import ExitStack

    import concourse.bass as bass
    import concourse.tile as tile
    from concourse import bacc, mybir
    from concourse.masks import make_identity

    f32 = mybir.dt.float32
    bf16 = mybir.dt.bfloat16
    i32 = mybir.dt.int32
    A = mybir.AluOpType
    AF = mybir.ActivationFunctionType

    c_pad = n_chunks * P
    n_bands = (n_chunks + 3) // 4        # bands of up to 4 chunks

    nc = bacc.Bacc("TRN2", target_bir_lowering=False, debug=False,
                   num_devices=N_CORES)

    x_d = nc.dram_tensor("x", [N, D], f32, kind="ExternalInput")
    # W arrives PRE-CAST to bf16 (host-side RNE cast, numerically
    # identical to the f32->bf16 cast-DMA it replaces): halves the
    # dominant HBM read stream (25.7 MB -> 12.8 MB per core)
    w_d = nc.dram_tensor("w", [c_pad, D], bf16, kind="ExternalInput")
    gidx_d = nc.dram_tensor("gidx", [P, 4], i32, kind="ExternalInput")
    out_dt = bf16 if out_bf16 else f32
    # transposed output: row = class (shard-local), col = batch slot j
    # (j = a*128 + p  <->  batch row n = 4p + a; host unpermutes)
    out_d = nc.dram_tensor("out", [c_pad * N], out_dt, kind="ExternalOutput")
    out2_d = nc.dram_tensor("out2", [P, 4], f32, kind="ExternalOutput")

    outT = out_d.ap().rearrange("(c n) -> c n", n=N)

    # W DMA loads cover a GROUP of bands at a time; within a group
    # partition p holds apg CONSECUTIVE weight rows -> contiguous DMA on
    # both sides, and the transposed-output DMA lands rows directly in
    # class order. First NHW groups are single-band so the pipeline
    # fills fast.
    GBANDS = 4
    NHW = 4                           # leading single-band groups
    group_bands = [[i] for i in range(NHW)]
    bb = NHW
    while bb < n_bands - 2:
        group_bands.append(list(range(bb, min(bb + GBANDS, n_bands - 2))))
        bb += GBANDS
    for bb in range(max(NHW, n_bands - 2), n_bands):
        group_bands.append([bb])     # small tail groups drain fast
    n_groups = len(group_bands)
    band_to_group = {}
    for gi, bl in enumerate(group_bands):
        for bj in bl:
            band_to_group[bj] = gi

    def group_rows(g):
        bl = group_bands[g]
        r0 = bl[0] * 512
        last = min((bl[-1] + 1) * 4, n_chunks) * P
        return r0, last - r0

    with tile.TileContext(nc) as tc:
        with ExitStack() as ctx:
            const_p = ctx.enter_context(tc.tile_pool(name="const", bufs=1))
            xp = ctx.enter_context(tc.tile_pool(name="xp", bufs=1))
            wl_p = ctx.enter_context(tc.tile_pool(name="wl", bufs=4))
            wls_p = ctx.enter_context(tc.tile_pool(name="wls", bufs=4))
            wsc_p = ctx.enter_context(tc.tile_pool(name="wsc", bufs=6))
            wst_p = ctx.enter_context(tc.tile_pool(name="wst", bufs=8))
            wtb_p = ctx.enter_context(tc.tile_pool(name="wtb", bufs=6))
            ob_p = ctx.enter_context(tc.tile_pool(name="ob", bufs=3))
            fix_p = ctx.enter_context(tc.tile_pool(name="fix", bufs=1))
            ptr_p = ctx.enter_context(
                tc.tile_pool(name="ptr", bufs=3, space="PSUM"))
            pmm_p = ctx.enter_context(
                tc.tile_pool(name="pmm", bufs=5, space="PSUM"))

            # ---------------- X load ----------------
            # FIRST op on the gpsimd SWDGE ring: strict FIFO puts X's
            # descriptors ahead of every W load -> lands ~10us.  (On the
            # sync ring it loses the engine round-robin to the W stream
            # and lands ~17us.)
            # partition p <- batch rows 4p..4p+3 (contiguous 8KB descs)
            xin = xp.tile([P, 4 * D], dtype=f32)    # row a at cols a*512
            # two half-loads: prep on rows a=0,1 starts ~1.2us before
            # the full tensor lands
            xsrc = x_d.ap().rearrange("(p a) d -> p a d", p=P)
            with tc.high_priority():
                nc.gpsimd.dma_start(
                    out=xin[:].rearrange("p (a d) -> p a d", d=D)
                        [:, 0:2, :],
                    in_=xsrc[:, 0:2, :])
                nc.gpsimd.dma_start(
                    out=xin[:].rearrange("p (a d) -> p a d", d=D)
                        [:, 2:4, :],
                    in_=xsrc[:, 2:4, :])

            # ACT warmup: pull the sqrt/square table-set loads off the
            # critical path.  Source tile comes from a DVE memset (NOT
            # ident -- that is built late on the gpsimd queue and the
            # scheduler would defer the warmup behind it).
            actw = const_p.tile([P, 1], dtype=f32)
            nc.vector.memset(actw[:], 1.0)
            nc.scalar.activation(out=actw[:], in_=actw[:], func=AF.Square)
            nc.scalar.activation(out=actw[:], in_=actw[:], func=AF.Sqrt)

            # cast (DVE) runs first; row-sumsq split: a=0,1 on DVE (stt
            # from the bf16 copy), a=2,3 on ACT (Square from f32)
            xsc = xp.tile([P, 4 * D], dtype=bf16)   # S/||x|| * X, bf16
            xss = xp.tile([P, 4], dtype=f32)
            xqa = xp.tile([P, D], dtype=bf16)       # ACT square scratch
            xqs = xp.tile([P, D], dtype=bf16)       # DVE square scratch
            # sumsq split 2 ACT (from f32, parallel with the casts) +
            # 2 DVE (bf16 stt after the casts): shortest serial chain
            for a in range(4):
                nc.vector.tensor_copy(
                    xsc[:, a * D:(a + 1) * D], xin[:, a * D:(a + 1) * D])
            for a in range(1, 4):
                nc.scalar.activation(
                    out=xqa[:], in_=xin[:, a * D:(a + 1) * D],
                    func=AF.Square, accum_out=xss[:, a:a + 1])
            for a in range(1):
                xs = xsc[:, a * D:(a + 1) * D]
                nc.vector.scalar_tensor_tensor(
                    out=xqs[:], in0=xs, scalar=1.0, in1=xs,
                    op0=A.mult, op1=A.mult, accum_out=xss[:, a:a + 1])
            xrs = xp.tile([P, 4], dtype=f32)      # 1/sumsq
            xrn = xp.tile([P, 4], dtype=f32)      # 1/||x||   (fixup)
            xrnS = xp.tile([P, 4], dtype=f32)     # S/||x||
            nc.vector.reciprocal(out=xrs[:], in_=xss[:])
            nc.scalar.sqrt(out=xrn[:], in_=xrs[:])
            nc.scalar.activation(out=xrnS[:], in_=xrs[:], func=AF.Sqrt,
                                 scale=S * S)
            for a in range(4):
                nc.vector.tensor_scalar_mul(
                    xsc[:, a * D:(a + 1) * D],
                    xsc[:, a * D:(a + 1) * D], xrnS[:, a:a + 1])

            # ---------------- W load (SWDGE DMA, cast f32 -> bf16) ------
            groups = {}     # g -> wl tile [P, GBANDS*4*D] bf16

            def emit_load_group(g):
                r0, rows = group_rows(g)
                apg = rows // P          # consecutive rows per partition
                if apg <= 4:
                    wl = wls_p.tile([P, 4 * D], dtype=bf16, tag="wls",
                                    name=f"wl{g}")
                else:
                    wl = wl_p.tile([P, GBANDS * 4 * D], dtype=bf16,
                                   tag="wl", name=f"wl{g}")
                nc.gpsimd.dma_start(
                    out=wl[:, :apg * D],
                    in_=w_d.ap()[r0:r0 + rows, :]
                        .rearrange("(p a) d -> p (a d)", p=P))
                groups[g] = wl

            # ---------------- per-BAND norm stats ----------------------
            # sumsq mostly on DVE (bf16 stt); a slice on ACT for
            # balance.  Emitted per band, 2 bands ahead of use, AFTER
            # that band's transpose drains: a stats op never sits in an
            # engine FIFO ahead of nearer-term work while waiting on a
            # far-future DMA (head-of-line blocking).
            bstats = {}      # b -> (wss, wrs, wrn) tiles [P, 4] f32

            def _bstat_tiles(b):
                if b not in bstats:
                    bstats[b] = (
                        wst_p.tile([P, 4], dtype=f32, tag="wss",
                                   name=f"wss{b}"),
                        wst_p.tile([P, 4], dtype=f32, tag="wrs",
                                   name=f"wrs{b}"),
                        wst_p.tile([P, 4], dtype=f32, tag="wrn",
                                   name=f"wrn{b}"))
                return bstats[b]

            def stage1_chunk(b, s):
                """Sumsq+rsqrt for one chunk (fine-grained: fill phase)."""
                g = band_to_group[b]
                wl = groups[g]
                sg = (b - group_bands[g][0]) * 4 + s
                wss, wrs, wrn = _bstat_tiles(b)
                wsl = wl[:, sg * D:(sg + 1) * D]
                wsq = wsc_p.tile([P, D], dtype=bf16, tag="wsq",
                                 name=f"wsq{b}_{s}")
                # scalar=xt[:,0:1] with op0=bypass: numerically inert,
                # but makes the stat depend on xt so the scheduler can
                # never sort it ahead of the X chain in the DVE FIFO
                nc.vector.scalar_tensor_tensor(
                    out=wsq[:], in0=wsl, scalar=xt[:, 0:1], in1=wsl,
                    op0=A.bypass, op1=A.mult, accum_out=wss[:, s:s + 1])
                nc.vector.reciprocal(out=wrs[:, s:s + 1],
                                     in_=wss[:, s:s + 1])
                nc.scalar.activation(out=wrn[:, s:s + 1],
                                     in_=wrs[:, s:s + 1], func=AF.Sqrt)

            def stage1_band(b):
                g = band_to_group[b]
                wl = groups[g]
                goff = (b - group_bands[g][0]) * 4
                nsub = min((b + 1) * 4, n_chunks) - b * 4
                wss, wrs, wrn = _bstat_tiles(b)
                for s in range(nsub):
                    sg = goff + s
                    wsl = wl[:, sg * D:(sg + 1) * D]
                    if (b * 4 + s) % 8 != 7:
                        wsq = wsc_p.tile([P, D], dtype=bf16, tag="wsq",
                                         name=f"wsq{b}_{s}")
                        nc.vector.scalar_tensor_tensor(
                            out=wsq[:], in0=wsl, scalar=1.0,
                            in1=wsl, op0=A.mult, op1=A.mult,
                            accum_out=wss[:, s:s + 1])
                    else:
                        wsqa = wsc_p.tile([P, D], dtype=bf16, tag="wsqa",
                                          name=f"wsqa{b}_{s}")
                        nc.scalar.activation(
                            out=wsqa[:], in_=wsl, func=AF.Square,
                            accum_out=wss[:, s:s + 1])
                nc.vector.reciprocal(out=wrs[:, :nsub], in_=wss[:, :nsub])
                nc.scalar.activation(out=wrn[:, :nsub], in_=wrs[:, :nsub],
                                     func=AF.Sqrt)

            def emit_band_transposes(b, gwl, goff, nsub):
                """PE-transpose one band's W chunks -> k-major wtb tile."""
                wtb = wtb_p.tile([P, 4 * 512], dtype=bf16, tag="wtb",
                                 name=f"wtb{b}")
                for s0 in range(0, nsub, 2):
                    wtp = ptr_p.tile([P, 8 * P], dtype=bf16, space="PSUM",
                                     tag="tp")
                    for ds in range(2):
                        si = goff + s0 + ds
                        for k in range(4):
                            nc.tensor.transpose(
                                out=wtp[:, k * 2 * P + ds * P:
                                        k * 2 * P + (ds + 1) * P],
                                in_=gwl[:, si * D + k * P:
                                        si * D + (k + 1) * P],
                                identity=ident[:])
                    # drain psum -> band tile (k-major layout)
                    nc.vector.tensor_copy(
                        out=wtb[:].rearrange("p (k c) -> p k c", k=4)
                            [:, :, s0 * P:(s0 + 2) * P],
                        in_=wtp[:].rearrange("p (k c) -> p k c", k=4))
                return wtb

            ident = const_p.tile([P, P], dtype=bf16)
            make_identity(nc, ident[:])

            # singles + first quad onto the SWDGE ring
            for _g in range(NHW + 1):
                emit_load_group(_g)

            # band 0's W transposes ahead of the X transposes in the PE
            # FIFO: they run while X-prep is still on DVE/ACT, so the
            # first matmul is gated only by xt
            wtb0 = emit_band_transposes(0, groups[0], 0, 4)

            # XT: [d(part), k-major: k*512 + j] bf16 (scaled), j=a*128+p
            xt = xp.tile([P, 4 * N], dtype=bf16)
            for k in range(4):
                pk = ptr_p.tile([P, 4 * P], dtype=bf16, space="PSUM", tag="tp")
                for a in range(4):
                    nc.tensor.transpose(
                        out=pk[:, a * P:(a + 1) * P],
                        in_=xsc[:, a * D + k * P: a * D + (k + 1) * P],
                        identity=ident[:])
                # drain on ACT (idle after the X squares): shortens the
                # DVE-serial head chain by ~2us
                nc.scalar.mul(out=xt[:, k * N:(k + 1) * N], in_=pk[:],
                              mul=1.0)

            # ---------------- sparse margin fixup (emitted mid-stream) ---
            fixst = {"vals": None}

            def emit_fixup_a():
                gidx = fix_p.tile([P, 4], dtype=i32)
                nc.sync.dma_start(out=gidx[:], in_=gidx_d.ap())

                wg = fix_p.tile([P, 4 * D], dtype=bf16)
                if fixup != "nogather":
                    for g in range(4):
                        nc.gpsimd.indirect_dma_start(
                            out=wg[:, g * D:(g + 1) * D], out_offset=None,
                            in_=w_d.ap(),
                            in_offset=bass.IndirectOffsetOnAxis(
                                ap=gidx[:, g:g + 1], axis=0))
                else:
                    nc.gpsimd.memset(wg[:], 1.0)
                fixst["wg"] = wg

            def emit_fixup_b(g):
                wg = fixst["wg"]
                if g == 0:
                    fixst["st"] = fix_p.tile([P, 16], dtype=f32,
                                             name="fixstat")
                st = fixst["st"]
                sumsq = st[:, 0:4]
                wgsq = fix_p.tile([P, D], dtype=f32, tag="wgsq",
                                  name=f"wgsq{g}")
                nc.scalar.activation(out=wgsq[:],
                                     in_=wg[:, g * D:(g + 1) * D],
                                     func=AF.Square,
                                     accum_out=sumsq[:, g:g + 1])
                dsc = fix_p.tile([P, D], dtype=f32, tag="wgsq",
                                 name=f"dsc{g}")
                nc.vector.tensor_tensor(
                    out=dsc[:], in0=xin[:, g * D:(g + 1) * D],
                    in1=wg[:, g * D:(g + 1) * D], op=A.mult)
                nc.vector.tensor_reduce(
                    out=st[:, 12 + g:13 + g], in_=dsc[:],
                    axis=mybir.AxisListType.X, op=A.add)

            def emit_fixup():
                st = fixst["st"]
                sumsq = st[:, 0:4]
                rs = st[:, 4:8]
                rn = st[:, 8:12]
                nc.vector.reciprocal(out=rs[:], in_=sumsq[:])
                nc.scalar.sqrt(out=rn[:], in_=rs[:])       # 1/||w||
                dots = st[:, 12:16]

                ft = fix_p.tile([P, 4 * 8], dtype=f32)
                cosv, cos2, sine, phi, alt, _unused, fvals, tmp = (
                    ft[:, i * 4:(i + 1) * 4] for i in range(8))
                mask_t = fix_p.tile([P, 4], dtype=mybir.dt.uint8)
                mask = mask_t[:]
                nc.vector.tensor_tensor(out=cosv, in0=dots[:], in1=rn[:],
                                        op=A.mult)
                nc.vector.tensor_tensor(out=cosv, in0=cosv, in1=xrn[:],
                                        op=A.mult)
                nc.vector.tensor_tensor(out=cos2, in0=cosv, in1=cosv,
                                        op=A.mult)
                nc.vector.tensor_scalar_min(cos2, cos2, 1.0)
                nc.scalar.activation(out=sine, in_=cos2, func=AF.Sqrt,
                                     scale=-1.0, bias=1.0)
                nc.vector.tensor_scalar_mul(phi, cosv, COS_M)
                nc.vector.tensor_scalar_mul(tmp, sine, SIN_M)
                nc.vector.tensor_tensor(out=phi, in0=phi, in1=tmp,
                                        op=A.subtract)
                nc.vector.tensor_scalar_add(alt, cosv, -MM)
                nc.vector.tensor_scalar(out=mask, in0=cosv, scalar1=TH,
                                        scalar2=None, op0=A.is_gt)
                nc.vector.select(out=fvals, mask=mask, on_true=phi,
                                 on_false=alt)
                nc.vector.tensor_scalar_mul(fvals, fvals, S)
                nc.sync.dma_start(out=out2_d.ap(), in_=fvals)
                fixst["vals"] = fvals


            # ---------------- main band loop ----------------
            ost = {}       # g -> staging tile [P, 8*N] bf16
            prog = {"load": NHW + 1}

            for b in range(n_bands):
                g = band_to_group[b]
                t = b - group_bands[g][0]
                chunks = range(b * 4, min((b + 1) * 4, n_chunks))
                nsub = len(chunks)

                # prefetch: W DMA ~12 bands ahead
                while (prog["load"] < n_groups
                       and group_bands[prog["load"]][0] <= b + 12):
                    emit_load_group(prog["load"])
                    prog["load"] += 1
                if t == 0:
                    ost[g] = ob_p.tile([P, GBANDS * 4 * N], dtype=out_dt,
                                       tag="ost", name=f"ost{g}")
                # fixup AFTER all W loads are emitted (b=12): the gather
                # descriptors are slow random-row reads; placed mid-ring
                # they block the quad stream for ~10us+
                if fixup != "none" and n_bands > 22:
                    if b == 12:
                        emit_fixup_a()
                    if 16 <= b <= 19:
                        emit_fixup_b(b - 16)
                    elif b == 21:
                        emit_fixup()

                gwl = groups[g]
                goff = (b - group_bands[g][0]) * 4

                # transpose RAW W chunks (pairs share one PSUM tile);
                # band 0's were pre-emitted ahead of the X-prep chain
                if b == 0:
                    wtb = wtb0
                else:
                    wtb = emit_band_transposes(b, gwl, goff, nsub)

                # stats lookahead 2 bands, EXCEPT the first band of each
                # group: lookahead 1 (prev group's last band), so the
                # ACT sqrt never queues ahead of drains while its
                # group's DMA is still in flight (head-of-line block).
                # bands 0/1 get per-chunk stats inline below.
                for m in range(b + 1, min(b + 3, n_bands)):
                    if m < 2:
                        continue
                    trig = max(m - 2, group_bands[band_to_group[m]][0] - 1)
                    if trig == b:
                        stage1_band(m)
                wrn = _bstat_tiles(b)[2]

                # matmuls: psumT[c, n] per chunk, accumulate over k
                for s in range(nsub):
                    sg = t * 4 + s
                    if b < 2:
                        stage1_chunk(b, s)
                    pm = pmm_p.tile([P, N], dtype=f32, space="PSUM")
                    for k in range(4):
                        nc.tensor.matmul(
                            out=pm[:],
                            lhsT=wtb[:, k * 512 + s * P: k * 512 + (s + 1) * P],
                            rhs=xt[:, k * N:(k + 1) * N],
                            start=(k == 0), stop=(k == 3))
                    # drain with per-partition 1/||w_c|| scale (all ACT;
                    # DVE owns sumsq + transpose drains).  Tail singles:
                    # alternate ACT/DVE so the last band drains ~2x
                    # faster (shorter kernel tail).
                    if b >= n_bands - 2 and s % 2 == 1:
                        nc.vector.tensor_scalar_mul(
                            ost[g][:, sg * N:(sg + 1) * N],
                            pm[:], wrn[:, s:s + 1])
                    else:
                        nc.scalar.mul(
                            out=ost[g][:, sg * N:(sg + 1) * N],
                            in_=pm[:], mul=wrn[:, s:s + 1])

                last_quad = (g == n_groups - 3)
                if last_quad:
                    # last quad: out-DMA per BAND (0.5 MB as soon as
                    # each band drains) to shorten the kernel tail.
                    # Band t's class rows are r0+p*apg+(4t..4t+nsub):
                    # slice the group-rearranged AP by column range.
                    r0, rows = group_rows(g)
                    nc.gpsimd.dma_start(
                        out=outT[r0:r0 + rows, :]
                            .rearrange("(p a) n -> p (a n)", p=P)
                            [:, t * 4 * N:(t * 4 + nsub) * N],
                        in_=ost[g][:, t * 4 * N:(t * 4 + nsub) * N])
                    if b == group_bands[g][-1]:
                        del ost[g]
                elif b == group_bands[g][-1]:
                    r0, rows = group_rows(g)
                    apg = rows // P
                    # out-DMA on the SAME SWDGE ring as the W loads:
                    # on the sync ring it steals the engine round-robin
                    # from the W stream (~270 GB/s effective vs ~390)
                    nc.gpsimd.dma_start(
                        out=outT[r0:r0 + rows, :]
                            .rearrange("(p a) n -> p (a n)", p=P),
                        in_=ost[g][:, :apg * N])
                    del ost[g]

            # margin values for tiny configs (normally emitted mid-stream)
            if fixup != "none" and fixst["vals"] is None:
                emit_fixup_a()
                for g in range(4):
                    emit_fixup_b(g)
                emit_fixup()

    nc.compile()
    return nc


def make_in_maps(input, label, weight, n_chunks=98, c_per=C_PER):
    """Shard the full inputs into per-core input maps."""
    from ml_dtypes import bfloat16

    c_pad = n_chunks * P
    x = np.ascontiguousarray(input, dtype=np.float32)
    lab = np.asarray(label).astype(np.int64)
    w = np.asarray(weight, dtype=np.float32)
    in_maps = []
    for i in range(N_CORES):
        c0 = i * c_per
        # pre-cast to bf16 on host (RNE, same numerics as the cast-DMA
        # it replaces); halves the device's W read bytes
        wi = np.empty((c_pad, D), dtype=bfloat16)
        wi[:c_per] = w[c0:c0 + c_per].astype(bfloat16)
        wi[c_per:] = 1.0
        loc = lab - c0
        valid = (loc >= 0) & (loc < c_per)
        g_rows = np.where(valid, loc, 0).astype(np.int32)
        in_maps.append({
            "x": x,
            "w": wi,
            # device row (p, a) = batch row 4p+a
            "gidx": np.ascontiguousarray(g_rows.reshape(P, 4)),
        })
    return in_maps


def kernel(input, label, weight):
    """Full inputs in, full output out. Runs SPMD on 8 NeuronCores."""
    _ensure_ntff_hook()
    from concourse.bass_utils import run_bass_kernel_spmd

    if "nc" not in _cache:
        _cache["nc"] = build_nc()
    nc = _cache["nc"]

    in_maps = make_in_maps(input, label, weight)
    res = run_bass_kernel_spmd(nc, in_maps, list(range(N_CORES)))
    _cache["last_result"] = res

    c_pad = 98 * P
    # device output is transposed: [c_pad, N] rows in class order; its
    # column j = a*128+p holds batch row n = 4p+a -> gather cols in
    # batch order while transposing (one fused fancy-index copy)
    big = np.concatenate(
        [res.results[i]["out"].reshape(c_pad, N)[:C_PER, :]
         for i in range(N_CORES)], axis=0)
    n_idx = np.arange(N)
    j_of_n = (n_idx % 4) * P + n_idx // 4
    out = big.T[j_of_n].astype(np.float32)
    out = np.ascontiguousarray(out)
    # place the device-computed margin values at the label positions
    lab = np.asarray(label).astype(np.int64)
    rows = np.arange(N)
    for i in range(N_CORES):
        vals = np.asarray(res.results[i]["out2"]).reshape(N)  # [p,a]->4p+a
        sel = (lab >= i * C_PER) & (lab < (i + 1) * C_PER)
        out[rows[sel], lab[sel]] = vals[sel]
    return out
